# revision 7
# baseline (speedup 1.0000x reference)
"""KAN layer kernel for 8 Trainium2 NeuronCores.

Math (reference):
    basis[b,i] = sum_h silu(x[b,i]*w1[i%K,h] + b1[i%K,h]) * w2[i%K,h] + b2[i%K]
    out[b,o]   = sum_i basis[b,i] * Wsum[o,i],   Wsum = W.sum(-1)   # [O,I]

Sharding: data-parallel over the input-feature axis I (16384 -> 8 x 2048).
Each core computes a partial out[64,1024] over its feature slice; host sums.

Per-core device program (memory-bound on reading its W slice):
  - W is host-cast to bf16 (tolerance 2e-2 >> bf16 error ~3e-3), halving
    HBM traffic vs fp32. Layout Wt[i, (k,o)]: 16 plain HWDGE loads of
    [128, 5120] on the Sync queue -- no DMA-accum, no SWDGE.
  - The k-reduction rides the PE: out[b,o] = sum_{i,k} basis[b,i]*W[o,i,k],
    so each i-tile issues 5 (k) x 2 (O-half) matmuls reusing the same
    lhsT = basisT tile, accumulating all 80 into each PSUM bank.
  - basis is computed with i on partitions in 6 wide ops per i-tile
    (vs 32 narrow ones): two DVE broadcast tensor_tensor (x*w1+b1 over
    [128,64,16]), one big ACT silu, one DVE w2 mult, one DVE h-reduce,
    one DVE +b2/cast-to-bf16.
"""
import numpy as np

B, I, O, K, H = 64, 16384, 1024, 5, 16
NCORES = 8
IC = I // NCORES          # 2048 features per core
P = 128                   # partition tile
NT = IC // P              # 16 i-tiles per core
NB = B                    # 64
NO = O                    # 1024
ROW = K * NO              # 5120 bf16 per Wt row
# cb16 (bf16) column blocks: xs [NT*NB] | w1 [NT*H] | b1 [NT*H] | w2 [NT*H]
X0 = NT * NB
X1 = X0 + NT * H
X2 = X1 + NT * H
CBW = X2 + NT * H

TRACE = False             # test.py sets True to capture an NTFF profile
LAST_RESULT = None


def _build():
    from contextlib import ExitStack
    from concourse import bacc, mybir, tile

    f32 = mybir.dt.float32
    bf16 = mybir.dt.bfloat16
    nc = bacc.Bacc("TRN2", target_bir_lowering=False, debug=False,
                   num_devices=NCORES)
    Wt = nc.declare_dram_parameter("Wt", [IC, ROW], bf16, isOutput=False)
    cb16d = nc.declare_dram_parameter("cb16", [P, CBW], bf16, isOutput=False)
    cb32d = nc.declare_dram_parameter("cb32", [P, NT], f32, isOutput=False)
    out = nc.declare_dram_parameter("out", [NB, NO], f32, isOutput=True)

    with tile.TileContext(nc) as tc, ExitStack() as ctx:
        const = ctx.enter_context(tc.tile_pool(name="const", bufs=1))
        wpool = ctx.enter_context(tc.tile_pool(name="w", bufs=6))
        ppool = ctx.enter_context(tc.tile_pool(name="pre", bufs=3))
        spool = ctx.enter_context(tc.tile_pool(name="silu", bufs=2))
        mpool = ctx.enter_context(tc.tile_pool(name="msum", bufs=2))
        apool = ctx.enter_context(tc.tile_pool(name="acc", bufs=NT))
        opool = ctx.enter_context(tc.tile_pool(name="out", bufs=1))
        psum = ctx.enter_context(tc.tile_pool(name="psum", bufs=1, space="PSUM"))

        # cb loads go on the SAME sync queue as (and ahead of) the W loads:
        # on the scalar queue their descriptors end up behind several 1.3MB
        # W transfers in the shared DMA-engine pool, stalling basis compute.
        cb16 = const.tile([P, CBW], bf16)
        cb32 = const.tile([P, NT], f32)
        nc.sync.dma_start(cb16[:, :], cb16d[:, :])
        nc.sync.dma_start(cb32[:, :], cb32d[:, :])

        ps0 = psum.tile([NB, 512], f32, tag="ps0")
        ps1 = psum.tile([NB, 512], f32, tag="ps1")

        # ---- W tile loads: 16 plain bf16 DMAs on the Sync HWDGE queue ----
        wts = []
        for t in range(NT):
            wt = wpool.tile([P, ROW], bf16, tag="wt")
            nc.sync.dma_start(wt[:, :], Wt[t * P:(t + 1) * P, :])
            wts.append(wt)

        # ---- basisT[i,b] per i-tile (DVE/ACT only; no W dependency) ----
        accs = []
        for t in range(NT):
            xs = cb16[:, t * NB:(t + 1) * NB]              # [P, 64]
            w1s = cb16[:, X0 + t * H:X0 + (t + 1) * H]     # [P, 16]
            b1s = cb16[:, X1 + t * H:X1 + (t + 1) * H]
            w2s = cb16[:, X2 + t * H:X2 + (t + 1) * H]
            # pre-activation on the otherwise-idle GpSimd engine; DVE keeps
            # the post-silu half of the chain.
            pre = ppool.tile([P, NB, H], bf16)
            nc.gpsimd.tensor_tensor(
                pre[:, :, :],
                xs[:, :, None].to_broadcast([P, NB, H]),
                w1s[:, None, :].to_broadcast([P, NB, H]),
                mybir.AluOpType.mult)
            pre2 = ppool.tile([P, NB, H], bf16)
            nc.gpsimd.tensor_tensor(
                pre2[:, :, :], pre[:, :, :],
                b1s[:, None, :].to_broadcast([P, NB, H]),
                mybir.AluOpType.add)
            s = spool.tile([P, NB, H], bf16)
            nc.scalar.activation(s[:, :, :], pre2[:, :, :],
                                 mybir.ActivationFunctionType.Silu)
            sw = spool.tile([P, NB, H], bf16)
            nc.vector.tensor_tensor(
                sw[:, :, :], s[:, :, :],
                w2s[:, None, :].to_broadcast([P, NB, H]),
                mybir.AluOpType.mult)
            bsum = mpool.tile([P, NB], f32)
            nc.vector.tensor_reduce(bsum[:, :], sw[:, :, :],
                                    axis=mybir.AxisListType.X,
                                    op=mybir.AluOpType.add)
            acc = apool.tile([P, NB], bf16, tag="acc")
            nc.vector.tensor_scalar_add(acc[:, :], bsum[:, :],
                                        cb32[:, t:t + 1])
            accs.append(acc)

        # ---- partial matmuls: out[b,o] += sum_k basisT.T @ W[:,k,:] ----
        for t in range(NT):
            for k in range(K):
                first = (t == 0 and k == 0)
                last = (t == NT - 1 and k == K - 1)
                nc.tensor.matmul(ps0[:, :], accs[t][:, :],
                                 wts[t][:, k * NO:k * NO + 512],
                                 start=first, stop=last)
                nc.tensor.matmul(ps1[:, :], accs[t][:, :],
                                 wts[t][:, k * NO + 512:(k + 1) * NO],
                                 start=first, stop=last)

        out_sb = opool.tile([NB, NO], f32)
        nc.vector.tensor_copy(out_sb[:, 0:512], ps0[:, :])
        nc.vector.tensor_copy(out_sb[:, 512:1024], ps1[:, :])
        nc.sync.dma_start(out[:, :], out_sb[:, :])
    nc.compile()
    return nc


def kernel(x, w1, b1, w2, b2, W):
    global LAST_RESULT
    import ml_dtypes
    from concourse.bass_utils import run_bass_kernel_spmd

    bf16 = ml_dtypes.bfloat16
    x = np.asarray(x, dtype=np.float32)
    W = np.asarray(W, dtype=np.float32)
    w1 = np.asarray(w1, dtype=np.float32)
    b1 = np.asarray(b1, dtype=np.float32)
    w2 = np.asarray(w2, dtype=np.float32)
    b2 = np.asarray(b2, dtype=np.float32)

    # ---- host prep: W -> bf16 [I, K*O] (i-major rows, k-major in-row) ----
    Wb = W.astype(bf16).view(np.uint16)                # [O, I, K]
    Wt_full = np.ascontiguousarray(Wb.transpose(1, 2, 0))  # [I, K, O] u16
    Wt_full = Wt_full.reshape(I, ROW).view(bf16)

    idx = np.arange(I) % K
    w1e = w1[idx].astype(bf16)                         # [I, H]
    b1e = b1[idx].astype(bf16)
    w2e = w2[idx].astype(bf16)
    b2e = b2[idx].astype(np.float32)                   # [I]

    def swz(a, cols):
        # [IC, cols] -> SBUF layout [P, NT*cols] (tile-major along free dim)
        return np.ascontiguousarray(
            a.reshape(NT, P, cols).transpose(1, 0, 2).reshape(P, NT * cols))

    x_bf = x.astype(bf16)
    in_maps = []
    for c in range(NCORES):
        sl = slice(c * IC, (c + 1) * IC)
        xt = np.ascontiguousarray(x_bf[:, sl].T)       # [IC, NB] bf16
        cb16 = np.concatenate(
            [swz(xt, NB), swz(w1e[sl], H), swz(b1e[sl], H), swz(w2e[sl], H)],
            axis=1)
        cb32 = swz(b2e[sl][:, None], 1)                # [P, NT] f32
        in_maps.append({
            "Wt": np.ascontiguousarray(Wt_full[sl]),
            "cb16": np.ascontiguousarray(cb16),
            "cb32": np.ascontiguousarray(cb32),
        })

    nc = _build()
    res = run_bass_kernel_spmd(nc, in_maps, list(range(NCORES)), trace=TRACE)
    LAST_RESULT = res
    out = np.zeros((B, O), dtype=np.float32)
    for c in range(NCORES):
        out += res.results[c]["out"]
    return out


# revision 8
# speedup vs baseline: 1.0122x; 1.0122x over previous
"""KAN layer kernel for 8 Trainium2 NeuronCores.

Math (reference):
    basis[b,i] = sum_h silu(x[b,i]*w1[i%K,h] + b1[i%K,h]) * w2[i%K,h] + b2[i%K]
    out[b,o]   = sum_i basis[b,i] * Wsum[o,i],   Wsum = W.sum(-1)   # [O,I]

Sharding: data-parallel over the input-feature axis I (16384 -> 8 x 2048).
Each core computes a partial out[64,1024] over its feature slice; host sums.

Per-core device program (memory-bound on reading its W slice):
  - W is host-cast to bf16 (tolerance 2e-2 >> bf16 error ~3e-3), halving
    HBM traffic vs fp32. Layout Wt[i, (k,o)]: 16 plain HWDGE loads of
    [128, 5120] on the Sync queue -- no DMA-accum, no SWDGE.
  - The k-reduction rides the PE: out[b,o] = sum_{i,k} basis[b,i]*W[o,i,k],
    so each i-tile issues 5 (k) x 2 (O-half) matmuls reusing the same
    lhsT = basisT tile, accumulating all 80 into each PSUM bank.
  - basis is computed with i on partitions in 6 wide ops per i-tile
    (vs 32 narrow ones): two DVE broadcast tensor_tensor (x*w1+b1 over
    [128,64,16]), one big ACT silu, one DVE w2 mult, one DVE h-reduce,
    one DVE +b2/cast-to-bf16.
"""
import numpy as np

B, I, O, K, H = 64, 16384, 1024, 5, 16
NCORES = 8
IC = I // NCORES          # 2048 features per core
P = 128                   # partition tile
NT = IC // P              # 16 i-tiles per core
NB = B                    # 64
NO = O                    # 1024
ROW = K * NO              # 5120 bf16 per Wt row
# cb16 (bf16) column blocks: xs [NT*NB] | w1 [NT*H] | b1 [NT*H] | w2 [NT*H]
X0 = NT * NB
X1 = X0 + NT * H
X2 = X1 + NT * H
CBW = X2 + NT * H

TRACE = False             # test.py sets True to capture an NTFF profile
LAST_RESULT = None


def _build():
    from contextlib import ExitStack
    from concourse import bacc, mybir, tile

    f32 = mybir.dt.float32
    bf16 = mybir.dt.bfloat16
    nc = bacc.Bacc("TRN2", target_bir_lowering=False, debug=False,
                   num_devices=NCORES)
    Wt = nc.declare_dram_parameter("Wt", [IC, ROW], bf16, isOutput=False)
    cb16d = nc.declare_dram_parameter("cb16", [P, CBW], bf16, isOutput=False)
    cb32d = nc.declare_dram_parameter("cb32", [P, NT], f32, isOutput=False)
    out = nc.declare_dram_parameter("out", [NB, NO], f32, isOutput=True)

    with tile.TileContext(nc) as tc, ExitStack() as ctx:
        const = ctx.enter_context(tc.tile_pool(name="const", bufs=1))
        wpool = ctx.enter_context(tc.tile_pool(name="w", bufs=6))
        ppool = ctx.enter_context(tc.tile_pool(name="pre", bufs=3))
        spool = ctx.enter_context(tc.tile_pool(name="silu", bufs=2))
        mpool = ctx.enter_context(tc.tile_pool(name="msum", bufs=2))
        apool = ctx.enter_context(tc.tile_pool(name="acc", bufs=NT))
        opool = ctx.enter_context(tc.tile_pool(name="out", bufs=1))
        psum = ctx.enter_context(tc.tile_pool(name="psum", bufs=1, space="PSUM"))

        # cb loads go on the SAME sync queue as (and ahead of) the W loads:
        # on the scalar queue their descriptors end up behind several 1.3MB
        # W transfers in the shared DMA-engine pool, stalling basis compute.
        cb16 = const.tile([P, CBW], bf16)
        cb32 = const.tile([P, NT], f32)
        nc.sync.dma_start(cb16[:, :], cb16d[:, :])
        nc.sync.dma_start(cb32[:, :], cb32d[:, :])

        ps0 = psum.tile([NB, 512], f32, tag="ps0")
        ps1 = psum.tile([NB, 512], f32, tag="ps1")

        # ---- W tile loads: 16 plain bf16 DMAs on the Sync HWDGE queue ----
        wts = []
        for t in range(NT):
            wt = wpool.tile([P, ROW], bf16, tag="wt")
            nc.sync.dma_start(wt[:, :], Wt[t * P:(t + 1) * P, :])
            wts.append(wt)

        # ---- basisT[i,b] per i-tile (no W dependency) ----
        # DVE is ~2.4x faster per op than GpSimd, but GpSimd is otherwise
        # idle: it precomputes the pre-activations of the LAST NG tiles in
        # parallel while DVE runs the full chain for the first tiles, so
        # DVE only does the post-silu half there.
        NG = 8
        pre2s = [None] * NT
        for t in range(NT - NG, NT):
            xs = cb16[:, t * NB:(t + 1) * NB]
            w1s = cb16[:, X0 + t * H:X0 + (t + 1) * H]
            b1s = cb16[:, X1 + t * H:X1 + (t + 1) * H]
            pre_g = ppool.tile([P, NB, H], bf16, tag="pre_g", bufs=2)
            nc.gpsimd.tensor_tensor(
                pre_g[:, :, :],
                xs[:, :, None].to_broadcast([P, NB, H]),
                w1s[:, None, :].to_broadcast([P, NB, H]),
                mybir.AluOpType.mult)
            pre2_g = ppool.tile([P, NB, H], bf16, tag="pre2_g", bufs=NG)
            nc.gpsimd.tensor_tensor(
                pre2_g[:, :, :], pre_g[:, :, :],
                b1s[:, None, :].to_broadcast([P, NB, H]),
                mybir.AluOpType.add)
            pre2s[t] = pre2_g

        accs = []
        for t in range(NT):
            xs = cb16[:, t * NB:(t + 1) * NB]              # [P, 64]
            w1s = cb16[:, X0 + t * H:X0 + (t + 1) * H]     # [P, 16]
            b1s = cb16[:, X1 + t * H:X1 + (t + 1) * H]
            w2s = cb16[:, X2 + t * H:X2 + (t + 1) * H]
            if pre2s[t] is None:
                pre = ppool.tile([P, NB, H], bf16, tag="pre_v", bufs=2)
                nc.vector.tensor_tensor(
                    pre[:, :, :],
                    xs[:, :, None].to_broadcast([P, NB, H]),
                    w1s[:, None, :].to_broadcast([P, NB, H]),
                    mybir.AluOpType.mult)
                pre2 = ppool.tile([P, NB, H], bf16, tag="pre2_v", bufs=2)
                nc.vector.tensor_tensor(
                    pre2[:, :, :], pre[:, :, :],
                    b1s[:, None, :].to_broadcast([P, NB, H]),
                    mybir.AluOpType.add)
                pre2s[t] = pre2
            s = spool.tile([P, NB, H], bf16, tag="s", bufs=3)
            nc.scalar.activation(s[:, :, :], pre2s[t][:, :, :],
                                 mybir.ActivationFunctionType.Silu)
            sw = spool.tile([P, NB, H], bf16, tag="sw", bufs=2)
            nc.vector.tensor_tensor(
                sw[:, :, :], s[:, :, :],
                w2s[:, None, :].to_broadcast([P, NB, H]),
                mybir.AluOpType.mult)
            bsum = mpool.tile([P, NB], f32)
            nc.vector.tensor_reduce(bsum[:, :], sw[:, :, :],
                                    axis=mybir.AxisListType.X,
                                    op=mybir.AluOpType.add)
            acc = apool.tile([P, NB], bf16, tag="acc")
            nc.vector.tensor_scalar_add(acc[:, :], bsum[:, :],
                                        cb32[:, t:t + 1])
            accs.append(acc)

        # ---- partial matmuls: out[b,o] += sum_k basisT.T @ W[:,k,:] ----
        for t in range(NT):
            for k in range(K):
                first = (t == 0 and k == 0)
                last = (t == NT - 1 and k == K - 1)
                nc.tensor.matmul(ps0[:, :], accs[t][:, :],
                                 wts[t][:, k * NO:k * NO + 512],
                                 start=first, stop=last)
                nc.tensor.matmul(ps1[:, :], accs[t][:, :],
                                 wts[t][:, k * NO + 512:(k + 1) * NO],
                                 start=first, stop=last)

        out_sb = opool.tile([NB, NO], f32)
        nc.vector.tensor_copy(out_sb[:, 0:512], ps0[:, :])
        nc.vector.tensor_copy(out_sb[:, 512:1024], ps1[:, :])
        nc.sync.dma_start(out[:, :], out_sb[:, :])
    nc.compile()
    return nc


def kernel(x, w1, b1, w2, b2, W):
    global LAST_RESULT
    import ml_dtypes
    from concourse.bass_utils import run_bass_kernel_spmd

    bf16 = ml_dtypes.bfloat16
    x = np.asarray(x, dtype=np.float32)
    W = np.asarray(W, dtype=np.float32)
    w1 = np.asarray(w1, dtype=np.float32)
    b1 = np.asarray(b1, dtype=np.float32)
    w2 = np.asarray(w2, dtype=np.float32)
    b2 = np.asarray(b2, dtype=np.float32)

    # ---- host prep: W -> bf16 [I, K*O] (i-major rows, k-major in-row) ----
    Wb = W.astype(bf16).view(np.uint16)                # [O, I, K]
    Wt_full = np.ascontiguousarray(Wb.transpose(1, 2, 0))  # [I, K, O] u16
    Wt_full = Wt_full.reshape(I, ROW).view(bf16)

    idx = np.arange(I) % K
    w1e = w1[idx].astype(bf16)                         # [I, H]
    b1e = b1[idx].astype(bf16)
    w2e = w2[idx].astype(bf16)
    b2e = b2[idx].astype(np.float32)                   # [I]

    def swz(a, cols):
        # [IC, cols] -> SBUF layout [P, NT*cols] (tile-major along free dim)
        return np.ascontiguousarray(
            a.reshape(NT, P, cols).transpose(1, 0, 2).reshape(P, NT * cols))

    x_bf = x.astype(bf16)
    in_maps = []
    for c in range(NCORES):
        sl = slice(c * IC, (c + 1) * IC)
        xt = np.ascontiguousarray(x_bf[:, sl].T)       # [IC, NB] bf16
        cb16 = np.concatenate(
            [swz(xt, NB), swz(w1e[sl], H), swz(b1e[sl], H), swz(w2e[sl], H)],
            axis=1)
        cb32 = swz(b2e[sl][:, None], 1)                # [P, NT] f32
        in_maps.append({
            "Wt": np.ascontiguousarray(Wt_full[sl]),
            "cb16": np.ascontiguousarray(cb16),
            "cb32": np.ascontiguousarray(cb32),
        })

    nc = _build()
    res = run_bass_kernel_spmd(nc, in_maps, list(range(NCORES)), trace=TRACE)
    LAST_RESULT = res
    out = np.zeros((B, O), dtype=np.float32)
    for c in range(NCORES):
        out += res.results[c]["out"]
    return out


# revision 9
# speedup vs baseline: 1.1902x; 1.1758x over previous
"""KAN layer kernel for 8 Trainium2 NeuronCores.

Math (reference):
    basis[b,i] = sum_h silu(x[b,i]*w1[i%K,h] + b1[i%K,h]) * w2[i%K,h] + b2[i%K]
    out[b,o]   = sum_i basis[b,i] * Wsum[o,i],   Wsum = W.sum(-1)   # [O,I]

Sharding: data-parallel over the input-feature axis I (16384 -> 8 x 2048).
Each core computes a partial out[64,1024] over its feature slice; host sums.

Per-core device program (memory-bound on reading its W slice):
  - W is host-cast to bf16 (tolerance 2e-2 >> bf16 error ~3e-3), halving
    HBM traffic vs fp32. Layout Wt[i, (k,o)]: 16 plain HWDGE loads of
    [128, 5120] on the Sync queue -- no DMA-accum, no SWDGE.
  - The k-reduction rides the PE: out[b,o] = sum_{i,k} basis[b,i]*W[o,i,k],
    so each i-tile issues 5 (k) x 2 (O-half) matmuls reusing the same
    lhsT = basisT tile, accumulating all 80 into each PSUM bank.
  - basis is computed with i on partitions in 6 wide ops per i-tile
    (vs 32 narrow ones): two DVE broadcast tensor_tensor (x*w1+b1 over
    [128,64,16]), one big ACT silu, one DVE w2 mult, one DVE h-reduce,
    one DVE +b2/cast-to-bf16.
"""
import numpy as np

B, I, O, K, H = 64, 16384, 1024, 5, 16
NCORES = 8
IC = I // NCORES          # 2048 features per core
P = 128                   # partition tile
NT = IC // P              # 16 i-tiles per core
NB = B                    # 64
NO = O                    # 1024
ROW = K * NO              # 5120 bf16 per Wt row
# cb16 (bf16) column blocks: xs [NT*NB] | w1 [NT*H] | b1 [NT*H] | w2 [NT*H]
X0 = NT * NB
X1 = X0 + NT * H
X2 = X1 + NT * H
CBW = X2 + NT * H

TRACE = False             # test.py sets True to capture an NTFF profile
LAST_RESULT = None


def _build():
    from contextlib import ExitStack
    from concourse import bacc, mybir, tile

    f32 = mybir.dt.float32
    bf16 = mybir.dt.bfloat16
    nc = bacc.Bacc("TRN2", target_bir_lowering=False, debug=False,
                   num_devices=NCORES)
    Wt = nc.declare_dram_parameter("Wt", [IC, ROW], bf16, isOutput=False)
    cb16d = nc.declare_dram_parameter("cb16", [P, CBW], bf16, isOutput=False)
    cb32d = nc.declare_dram_parameter("cb32", [P, NT], f32, isOutput=False)
    out = nc.declare_dram_parameter("out", [NB, NO], f32, isOutput=True)

    with tile.TileContext(nc) as tc, ExitStack() as ctx:
        const = ctx.enter_context(tc.tile_pool(name="const", bufs=1))
        wpool = ctx.enter_context(tc.tile_pool(name="w", bufs=6))
        ppool = ctx.enter_context(tc.tile_pool(name="pre", bufs=3))
        spool = ctx.enter_context(tc.tile_pool(name="silu", bufs=2))
        mpool = ctx.enter_context(tc.tile_pool(name="msum", bufs=2))
        apool = ctx.enter_context(tc.tile_pool(name="acc", bufs=NT))
        opool = ctx.enter_context(tc.tile_pool(name="out", bufs=1))
        psum = ctx.enter_context(tc.tile_pool(name="psum", bufs=1, space="PSUM"))

        # cb loads go on the SAME sync queue as (and ahead of) the W loads:
        # on the scalar queue their descriptors end up behind several 1.3MB
        # W transfers in the shared DMA-engine pool, stalling basis compute.
        cb16 = const.tile([P, CBW], bf16)
        cb32 = const.tile([P, NT], f32)
        nc.sync.dma_start(cb16[:, :], cb16d[:, :])
        nc.sync.dma_start(cb32[:, :], cb32d[:, :])

        ps0 = psum.tile([NB, 512], f32, tag="ps0")
        ps1 = psum.tile([NB, 512], f32, tag="ps1")

        # ---- W tile loads: 16 plain bf16 DMAs on the Sync HWDGE queue ----
        wts = []
        for t in range(NT):
            wt = wpool.tile([P, ROW], bf16, tag="wt")
            nc.sync.dma_start(wt[:, :], Wt[t * P:(t + 1) * P, :])
            wts.append(wt)

        # ---- basisT[i,b] per i-tile (DVE/ACT only; no W dependency) ----
        # DVE's bf16 2x mode needs every operand's LAST AP dim packed
        # (stride 1). In the [P, b, h] layout the w1/b1/w2 broadcasts
        # qualify, but x broadcast over h does not -- so ACT (idle early)
        # materializes x replicated over h once per tile, making all three
        # DVE tensor_tensor ops 2x. The h-reduction runs as a 2x-eligible
        # add-tree (tensor_reduce has no 2x mode), with the last fold fused
        # with the +b2 / bf16 cast in one scalar_tensor_tensor.
        xreps = []
        for t in range(NT):
            xs = cb16[:, t * NB:(t + 1) * NB]              # [P, 64]
            xrep = ppool.tile([P, NB, H], bf16, tag="xrep", bufs=NT)
            nc.scalar.copy(xrep[:, :, :],
                           xs[:, :, None].to_broadcast([P, NB, H]))
            xreps.append(xrep)

        accs = []
        for t in range(NT):
            w1s = cb16[:, X0 + t * H:X0 + (t + 1) * H]     # [P, 16]
            b1s = cb16[:, X1 + t * H:X1 + (t + 1) * H]
            w2s = cb16[:, X2 + t * H:X2 + (t + 1) * H]
            pre = ppool.tile([P, NB, H], bf16, tag="pre", bufs=2)
            nc.vector.tensor_tensor(
                pre[:, :, :], xreps[t][:, :, :],
                w1s[:, None, :].to_broadcast([P, NB, H]),
                mybir.AluOpType.mult)
            pre2 = ppool.tile([P, NB, H], bf16, tag="pre2", bufs=2)
            nc.vector.tensor_tensor(
                pre2[:, :, :], pre[:, :, :],
                b1s[:, None, :].to_broadcast([P, NB, H]),
                mybir.AluOpType.add)
            s = spool.tile([P, NB, H], bf16, tag="s", bufs=3)
            nc.scalar.activation(s[:, :, :], pre2[:, :, :],
                                 mybir.ActivationFunctionType.Silu)
            sw = spool.tile([P, NB, H], bf16, tag="sw", bufs=2)
            nc.vector.tensor_tensor(
                sw[:, :, :], s[:, :, :],
                w2s[:, None, :].to_broadcast([P, NB, H]),
                mybir.AluOpType.mult)
            f1 = mpool.tile([P, NB, H // 2], bf16, tag="f1", bufs=2)
            nc.vector.tensor_tensor(
                f1[:, :, :], sw[:, :, 0:H // 2], sw[:, :, H // 2:H],
                mybir.AluOpType.add)
            f2 = mpool.tile([P, NB, H // 4], bf16, tag="f2", bufs=2)
            nc.vector.tensor_tensor(
                f2[:, :, :], f1[:, :, 0:H // 4], f1[:, :, H // 4:H // 2],
                mybir.AluOpType.add)
            f3 = mpool.tile([P, NB, 2], bf16, tag="f3", bufs=2)
            nc.vector.tensor_tensor(
                f3[:, :, :], f2[:, :, 0:2], f2[:, :, 2:4],
                mybir.AluOpType.add)
            acc = apool.tile([P, NB], bf16, tag="acc")
            # acc = (f3[...,0] + b2) + f3[...,1], cast to bf16
            nc.vector.scalar_tensor_tensor(
                acc[:, :], f3[:, :, 0], cb32[:, t:t + 1], f3[:, :, 1],
                op0=mybir.AluOpType.add, op1=mybir.AluOpType.add)
            accs.append(acc)

        # ---- partial matmuls: out[b,o] += sum_k basisT.T @ W[:,k,:] ----
        for t in range(NT):
            for k in range(K):
                first = (t == 0 and k == 0)
                last = (t == NT - 1 and k == K - 1)
                nc.tensor.matmul(ps0[:, :], accs[t][:, :],
                                 wts[t][:, k * NO:k * NO + 512],
                                 start=first, stop=last)
                nc.tensor.matmul(ps1[:, :], accs[t][:, :],
                                 wts[t][:, k * NO + 512:(k + 1) * NO],
                                 start=first, stop=last)

        out_sb = opool.tile([NB, NO], f32)
        nc.vector.tensor_copy(out_sb[:, 0:512], ps0[:, :])
        nc.vector.tensor_copy(out_sb[:, 512:1024], ps1[:, :])
        nc.sync.dma_start(out[:, :], out_sb[:, :])
    nc.compile()
    return nc


def kernel(x, w1, b1, w2, b2, W):
    global LAST_RESULT
    import ml_dtypes
    from concourse.bass_utils import run_bass_kernel_spmd

    bf16 = ml_dtypes.bfloat16
    x = np.asarray(x, dtype=np.float32)
    W = np.asarray(W, dtype=np.float32)
    w1 = np.asarray(w1, dtype=np.float32)
    b1 = np.asarray(b1, dtype=np.float32)
    w2 = np.asarray(w2, dtype=np.float32)
    b2 = np.asarray(b2, dtype=np.float32)

    # ---- host prep: W -> bf16 [I, K*O] (i-major rows, k-major in-row) ----
    Wb = W.astype(bf16).view(np.uint16)                # [O, I, K]
    Wt_full = np.ascontiguousarray(Wb.transpose(1, 2, 0))  # [I, K, O] u16
    Wt_full = Wt_full.reshape(I, ROW).view(bf16)

    idx = np.arange(I) % K
    w1e = w1[idx].astype(bf16)                         # [I, H]
    b1e = b1[idx].astype(bf16)
    w2e = w2[idx].astype(bf16)
    b2e = b2[idx].astype(np.float32)                   # [I]

    def swz(a, cols):
        # [IC, cols] -> SBUF layout [P, NT*cols] (tile-major along free dim)
        return np.ascontiguousarray(
            a.reshape(NT, P, cols).transpose(1, 0, 2).reshape(P, NT * cols))

    x_bf = x.astype(bf16)
    in_maps = []
    for c in range(NCORES):
        sl = slice(c * IC, (c + 1) * IC)
        xt = np.ascontiguousarray(x_bf[:, sl].T)       # [IC, NB] bf16
        cb16 = np.concatenate(
            [swz(xt, NB), swz(w1e[sl], H), swz(b1e[sl], H), swz(w2e[sl], H)],
            axis=1)
        cb32 = swz(b2e[sl][:, None], 1)                # [P, NT] f32
        in_maps.append({
            "Wt": np.ascontiguousarray(Wt_full[sl]),
            "cb16": np.ascontiguousarray(cb16),
            "cb32": np.ascontiguousarray(cb32),
        })

    nc = _build()
    res = run_bass_kernel_spmd(nc, in_maps, list(range(NCORES)), trace=TRACE)
    LAST_RESULT = res
    out = np.zeros((B, O), dtype=np.float32)
    for c in range(NCORES):
        out += res.results[c]["out"]
    return out


# revision 12
# speedup vs baseline: 1.3845x; 1.1633x over previous
"""KAN layer kernel for 8 Trainium2 NeuronCores.

Math (reference):
    basis[b,i] = sum_h silu(x[b,i]*w1[i%K,h] + b1[i%K,h]) * w2[i%K,h] + b2[i%K]
    out[b,o]   = sum_i basis[b,i] * Wsum[o,i],   Wsum = W.sum(-1)   # [O,I]

Sharding: data-parallel over the input-feature axis I (16384 -> 8 x 2048).
Each core computes a partial out[64,1024] over its feature slice; host sums.

Per-core device program (memory-bound on reading its W slice):
  - W is host-cast to bf16 (tolerance 2e-2 >> bf16 error ~3e-3), halving
    HBM traffic vs fp32. Layout Wt[i, (k,o)]: 16 plain HWDGE loads of
    [128, 5120] on the Sync queue -- no DMA-accum, no SWDGE.
  - The k-reduction rides the PE: out[b,o] = sum_{i,k} basis[b,i]*W[o,i,k],
    so each i-tile issues 5 (k) x 2 (O-half) matmuls reusing the same
    lhsT = basisT tile, accumulating all 80 into each PSUM bank.
  - basis is computed with i on partitions in 6 wide ops per i-tile
    (vs 32 narrow ones): two DVE broadcast tensor_tensor (x*w1+b1 over
    [128,64,16]), one big ACT silu, one DVE w2 mult, one DVE h-reduce,
    one DVE +b2/cast-to-bf16.
"""
import numpy as np

B, I, O, K, H = 64, 16384, 1024, 5, 16
NCORES = 8
IC = I // NCORES          # 2048 features per core
P = 128                   # partition tile
NT = IC // P              # 16 i-tiles per core
NB = B                    # 64
NO = O                    # 1024
ROW = K * NO              # 5120 bf16 per Wt row
# cb16 (bf16) column blocks: xs [NT*NB] | w1 [NT*H] | b1 [NT*H] | w2 [NT*H]
X0 = NT * NB
X1 = X0 + NT * H
X2 = X1 + NT * H
CBW = X2 + NT * H

TRACE = False             # test.py sets True to capture an NTFF profile
LAST_RESULT = None


def _build():
    from contextlib import ExitStack
    from concourse import bacc, mybir, tile

    f32 = mybir.dt.float32
    bf16 = mybir.dt.bfloat16
    nc = bacc.Bacc("TRN2", target_bir_lowering=False, debug=False,
                   num_devices=NCORES)
    Wt = nc.declare_dram_parameter("Wt", [IC, ROW], bf16, isOutput=False)
    cb16d = nc.declare_dram_parameter("cb16", [P, CBW], bf16, isOutput=False)
    cb32d = nc.declare_dram_parameter("cb32", [P, NT], f32, isOutput=False)
    out = nc.declare_dram_parameter("out", [NB, NO], f32, isOutput=True)

    with tile.TileContext(nc) as tc, ExitStack() as ctx:
        const = ctx.enter_context(tc.tile_pool(name="const", bufs=1))
        wpool = ctx.enter_context(tc.tile_pool(name="w", bufs=6))
        ppool = ctx.enter_context(tc.tile_pool(name="pre", bufs=3))
        spool = ctx.enter_context(tc.tile_pool(name="silu", bufs=2))
        mpool = ctx.enter_context(tc.tile_pool(name="msum", bufs=2))
        apool = ctx.enter_context(tc.tile_pool(name="acc", bufs=NT))
        opool = ctx.enter_context(tc.tile_pool(name="out", bufs=1))
        psum = ctx.enter_context(tc.tile_pool(name="psum", bufs=1, space="PSUM"))

        # cb loads go on the SAME sync queue as (and ahead of) the W loads:
        # on the scalar queue their descriptors end up behind several 1.3MB
        # W transfers in the shared DMA-engine pool, stalling basis compute.
        cb16 = const.tile([P, CBW], bf16)
        cb32 = const.tile([P, NT], f32)
        nc.sync.dma_start(cb16[:, :], cb16d[:, :])
        nc.sync.dma_start(cb32[:, :], cb32d[:, :])

        ps0 = psum.tile([NB, 512], f32, tag="ps0")
        ps1 = psum.tile([NB, 512], f32, tag="ps1")

        # ---- W tile loads: plain bf16 DMAs on the Sync HWDGE queue.
        # The last tile is split so its first matmuls start ~1.8us before
        # the final bytes land (shorter kernel tail). ----
        wts = []
        for t in range(NT):
            wt = wpool.tile([P, ROW], bf16, tag="wt")
            if t == NT - 1:
                nc.sync.dma_start(wt[:, 0:2 * NO], Wt[t * P:(t + 1) * P, 0:2 * NO])
                nc.sync.dma_start(wt[:, 2 * NO:ROW],
                                  Wt[t * P:(t + 1) * P, 2 * NO:ROW])
            else:
                nc.sync.dma_start(wt[:, :], Wt[t * P:(t + 1) * P, :])
            wts.append(wt)

        # ---- basisT[i,b] per i-tile (DVE/ACT only; no W dependency) ----
        # DVE's bf16 2x mode needs every operand's LAST AP dim packed
        # (stride 1). In the [P, b, h] layout the w1/b1/w2 broadcasts
        # qualify, but x broadcast over h does not -- so ACT (idle early)
        # materializes x replicated over h once per tile, making all three
        # DVE tensor_tensor ops 2x. The h-reduction runs as a 2x-eligible
        # add-tree (tensor_reduce has no 2x mode), with the last fold fused
        # with the +b2 / bf16 cast in one scalar_tensor_tensor.
        # Pin the silu_and_others table (it also serves Copy) with a dummy
        # silu so the whole kernel needs exactly one ACT_TABLE_LOAD.
        dummy = const.tile([1, 1], f32)
        nc.scalar.activation(dummy[:, :], cb32[0:1, 0:1],
                             mybir.ActivationFunctionType.Silu)

        xreps, pre2s, accs = [], [], []

        def emit_front(t):
            # ACT: materialize x broadcast over h; DVE: 2x-mode affine.
            xs = cb16[:, t * NB:(t + 1) * NB]              # [P, 64]
            w1s = cb16[:, X0 + t * H:X0 + (t + 1) * H]     # [P, 16]
            b1s = cb16[:, X1 + t * H:X1 + (t + 1) * H]
            xrep = ppool.tile([P, NB, H], bf16, tag="xrep", bufs=3)
            nc.scalar.copy(xrep[:, :, :],
                           xs[:, :, None].to_broadcast([P, NB, H]))
            xreps.append(xrep)
            pre = ppool.tile([P, NB, H], bf16, tag="pre", bufs=2)
            nc.vector.tensor_tensor(
                pre[:, :, :], xrep[:, :, :],
                w1s[:, None, :].to_broadcast([P, NB, H]),
                mybir.AluOpType.mult)
            pre2 = ppool.tile([P, NB, H], bf16, tag="pre2", bufs=3)
            nc.vector.tensor_tensor(
                pre2[:, :, :], pre[:, :, :],
                b1s[:, None, :].to_broadcast([P, NB, H]),
                mybir.AluOpType.add)
            pre2s.append(pre2)

        def emit_back(t):
            # ACT: silu; DVE: w2 mult + 2x add-tree + fused +b2/bf16 cast.
            w2s = cb16[:, X2 + t * H:X2 + (t + 1) * H]
            s = spool.tile([P, NB, H], bf16, tag="s", bufs=3)
            nc.scalar.activation(s[:, :, :], pre2s[t][:, :, :],
                                 mybir.ActivationFunctionType.Silu)
            sw = spool.tile([P, NB, H], bf16, tag="sw", bufs=2)
            nc.vector.tensor_tensor(
                sw[:, :, :], s[:, :, :],
                w2s[:, None, :].to_broadcast([P, NB, H]),
                mybir.AluOpType.mult)
            f1 = mpool.tile([P, NB, H // 2], bf16, tag="f1", bufs=2)
            nc.vector.tensor_tensor(
                f1[:, :, :], sw[:, :, 0:H // 2], sw[:, :, H // 2:H],
                mybir.AluOpType.add)
            f2 = mpool.tile([P, NB, H // 4], bf16, tag="f2", bufs=2)
            nc.vector.tensor_tensor(
                f2[:, :, :], f1[:, :, 0:H // 4], f1[:, :, H // 4:H // 2],
                mybir.AluOpType.add)
            f3 = mpool.tile([P, NB, 2], bf16, tag="f3", bufs=2)
            nc.vector.tensor_tensor(
                f3[:, :, :], f2[:, :, 0:2], f2[:, :, 2:4],
                mybir.AluOpType.add)
            acc = apool.tile([P, NB], bf16, tag="acc")
            # acc = (f3[...,0] + b2) + f3[...,1], cast to bf16
            nc.vector.scalar_tensor_tensor(
                acc[:, :], f3[:, :, 0], cb32[:, t:t + 1], f3[:, :, 1],
                op0=mybir.AluOpType.add, op1=mybir.AluOpType.add)
            accs.append(acc)

        # Software-pipelined emission: the front half (copy + affine) of
        # tile t+1 is emitted before the back half of tile t, so neither
        # engine ever waits on the other's just-issued work.
        emit_front(0)
        for t in range(1, NT):
            emit_front(t)
            emit_back(t - 1)
        emit_back(NT - 1)

        # ---- partial matmuls: out[b,o] += sum_k basisT.T @ W[:,k,:] ----
        # Last tile runs all ps0 matmuls before ps1's so the ps0 bank can
        # drain (copy + store) while ps1 is still accumulating.
        for t in range(NT - 1):
            for k in range(K):
                first = (t == 0 and k == 0)
                nc.tensor.matmul(ps0[:, :], accs[t][:, :],
                                 wts[t][:, k * NO:k * NO + 512],
                                 start=first, stop=False)
                nc.tensor.matmul(ps1[:, :], accs[t][:, :],
                                 wts[t][:, k * NO + 512:(k + 1) * NO],
                                 start=first, stop=False)
        tl = NT - 1
        out_sb = opool.tile([NB, NO], f32)
        for k in range(K):
            nc.tensor.matmul(ps0[:, :], accs[tl][:, :],
                             wts[tl][:, k * NO:k * NO + 512],
                             start=False, stop=(k == K - 1))
        nc.vector.tensor_copy(out_sb[:, 0:512], ps0[:, :])
        nc.sync.dma_start(out[:, 0:512], out_sb[:, 0:512])
        for k in range(K):
            nc.tensor.matmul(ps1[:, :], accs[tl][:, :],
                             wts[tl][:, k * NO + 512:(k + 1) * NO],
                             start=False, stop=(k == K - 1))
        nc.vector.tensor_copy(out_sb[:, 512:1024], ps1[:, :])
        nc.sync.dma_start(out[:, 512:1024], out_sb[:, 512:1024])
    nc.compile()
    return nc


def kernel(x, w1, b1, w2, b2, W):
    global LAST_RESULT
    import ml_dtypes
    from concourse.bass_utils import run_bass_kernel_spmd

    bf16 = ml_dtypes.bfloat16
    x = np.asarray(x, dtype=np.float32)
    W = np.asarray(W, dtype=np.float32)
    w1 = np.asarray(w1, dtype=np.float32)
    b1 = np.asarray(b1, dtype=np.float32)
    w2 = np.asarray(w2, dtype=np.float32)
    b2 = np.asarray(b2, dtype=np.float32)

    # ---- host prep: W -> bf16 [I, K*O] (i-major rows, k-major in-row) ----
    Wb = W.astype(bf16).view(np.uint16)                # [O, I, K]
    Wt_full = np.ascontiguousarray(Wb.transpose(1, 2, 0))  # [I, K, O] u16
    Wt_full = Wt_full.reshape(I, ROW).view(bf16)

    idx = np.arange(I) % K
    w1e = w1[idx].astype(bf16)                         # [I, H]
    b1e = b1[idx].astype(bf16)
    w2e = w2[idx].astype(bf16)
    b2e = b2[idx].astype(np.float32)                   # [I]

    def swz(a, cols):
        # [IC, cols] -> SBUF layout [P, NT*cols] (tile-major along free dim)
        return np.ascontiguousarray(
            a.reshape(NT, P, cols).transpose(1, 0, 2).reshape(P, NT * cols))

    x_bf = x.astype(bf16)
    in_maps = []
    for c in range(NCORES):
        sl = slice(c * IC, (c + 1) * IC)
        xt = np.ascontiguousarray(x_bf[:, sl].T)       # [IC, NB] bf16
        cb16 = np.concatenate(
            [swz(xt, NB), swz(w1e[sl], H), swz(b1e[sl], H), swz(w2e[sl], H)],
            axis=1)
        cb32 = swz(b2e[sl][:, None], 1)                # [P, NT] f32
        in_maps.append({
            "Wt": np.ascontiguousarray(Wt_full[sl]),
            "cb16": np.ascontiguousarray(cb16),
            "cb32": np.ascontiguousarray(cb32),
        })

    nc = _build()
    res = run_bass_kernel_spmd(nc, in_maps, list(range(NCORES)), trace=TRACE)
    LAST_RESULT = res
    out = np.zeros((B, O), dtype=np.float32)
    for c in range(NCORES):
        out += res.results[c]["out"]
    return out
